# revision 42
# baseline (speedup 1.0000x reference)
"""Trainium2 Bass kernel for policy-weighted multi-head attention.

Reference computation (per batch b, 8 batches):
    qkv = x @ qkv_w.T                     # [N, 3*H*HD]
    q, k, v per head                      # H=12 heads, HD=64
    s = (q * HD^-0.5) @ k.T               # [N, N]
    a[n,m] ~ exp(s[n,m]) * (pol[m] + (1-pol[m])*eye)  normalized over m
    out = a @ v ; y = out @ proj_w.T + b

Sharding: pure data parallel, one batch per NeuronCore (8 cores).

Kernel strategy (per core).  The governing constraint is that every
engine queue is in-order: a matmul whose semaphore is not yet satisfied
blocks everything queued behind it, and each such stall costs the PE
~200ns of pipeline refill on top of the wait.  The schedule therefore
(a) keeps every producer (exp on ACT, reciprocal/norm on DVE) well
ahead of its PE consumer and (b) emits PE work in same-chain runs of
4-8 matmuls (chain-type switches cost ~150-200ns each):
  - Everything the PE multiplies is bf16 (shipped bf16 from the host:
    halves startup HBM traffic, enables fast weight loads, 2x DVE on
    the PSUM->SBUF casts).  Accumulation stays f32 in PSUM; measured
    rel err 5.7e-3 vs the 2e-2 gate.
  - Host pre-packs x/weights into partition-major layouts so every SBUF
    load is one DMA descriptor; no on-chip transposes.  The first qk
    chain's two slices go first on separate DMA rings (xT on sync,
    wqk0 on scalar); qk chains run cc-outer/half-inner so links start
    as their xT slice lands.
  - Attention in the S^T layout (partitions = key index m): the softmax
    denominator folds into the a@v matmul via a pol column appended to V
    (V rows pre-scaled by pol[m]); the diagonal policy term is a masked
    multiply with a precomputed [128, 8, 128] mask (diag = 1/pol).
  - st/exp run at [128,2,512] pair granularity (one exp per 1024 cols).
    q and k live in separate bf16 tiles (distinct SBUF regions for the
    moving and LDWEIGHTS fetch streams).
  - The a@v matmuls of block b run one block LATE, in two runs of 4
    interleaved with block b+1's two st groups of 4: by then all E
    tiles exist, so av links are never exp-gated and pad the window
    until exp(t-1) frees the next st PSUM buf (2-buf rotation).  The
    last block drains its own av inline.  E pool holds 9 tiles (two
    blocks in flight).
  - Normalization per head PAIR, off the critical path: av is copied to
    SBUF at once (releasing its PSUM bank); the denominator rows land
    on partitions 0/32 of a pair tile via ACT Copy; reciprocal_approx_
    fast (~1/5 the DVE cost of reciprocal(), 18 bits) serves both
    heads; at norm time one masked-[33,128] matmul broadcasts both rcp
    rows into a [128,512] PSUM tile for one DVE multiply per pair.
    Norms pop from a pending queue ~2 pairs late.
  - Query-half-outer schedule: heads 0..11 on queries 0:512, then on
    512:1024 (rotated so pair 4 finishes last).  PE filler work rides in
    the holes the st->exp->av pipeline would otherwise leave: qk chains
    for pairs 3..5 fill half 0 (pairs 0..2 load in the prologue); the
    output projection fills half 1, ec-outer so norm-gated links come
    last, with bias folded in as a rank-1 first link and only pair 4's
    links deferred to the tail.  Fillers are strict-FIFO (they share
    PSUM) and readiness-gated so no instruction precedes its producer.
  - exp on Scalar (table pre-warmed), diag multiplies on GpSimd, PSUM
    drains (qk casts, av/y copies) and normalization on DVE; y DMAs
    alternate rings so tail completions overlap.
  - max-subtraction and the eps terms of the softmax are dropped: logits
    are ~N(0,1) so exp() cannot overflow; the eps corrections are ~1e-9.
"""

import os

os.environ.setdefault("JAX_PLATFORMS", "axon")

from contextlib import ExitStack

import ml_dtypes
import numpy as np

import concourse.bass as bass
import concourse.tile as tile
from concourse import bacc, mybir
from concourse.bass_utils import run_bass_kernel_spmd

B, N, C = 8, 1024, 768
H, HD = 12, 64
SCALE = HD ** (-0.5)
F32 = mybir.dt.float32
F32R = mybir.dt.float32r
BF16 = mybir.dt.bfloat16
P = 128
NC_ = N // P  # 8 seq chunks
CC = C // P  # 6 channel chunks / head pairs
NP = H // 2  # 6 head pairs
NH = 2  # query halves
HQ = N // NH  # 512

LAST_RESULTS = None  # BassKernelResults of the most recent run (for test.py)


def _build_nc():
    nc = bacc.Bacc(None, target_bir_lowering=False)

    xT_d = nc.dram_tensor("xT", [P, CC, N], BF16, kind="ExternalInput")
    wqk_d = nc.dram_tensor("wqk", [P, CC, NP, 2, P], BF16, kind="ExternalInput")
    wv_d = nc.dram_tensor("wv", [P, CC, C], BF16, kind="ExternalInput")
    pw_d = nc.dram_tensor("pw", [P, CC, C], BF16, kind="ExternalInput")
    bias_d = nc.dram_tensor("bias", [C], BF16, kind="ExternalInput")
    polT_d = nc.dram_tensor("polT", [P, NC_], F32, kind="ExternalInput")
    dmask_d = nc.dram_tensor("dmask", [P, NC_, P], BF16, kind="ExternalInput")
    y_d = nc.dram_tensor("y", [N, C], F32, kind="ExternalOutput")

    with ExitStack() as ctx:
        tc = ctx.enter_context(tile.TileContext(nc))

        persist = ctx.enter_context(tc.tile_pool(name="persist", bufs=1))
        # q and k in bf16, in SEPARATE tiles: halves the DVE cast cost and
        # SBUF footprint, enables FWL on the st stationary (128 cols, non-
        # fp32), and keeps the st moving + LDWEIGHTS fetch streams out of
        # the same SBUF region.
        q_sb = persist.tile([P, NP, N], BF16)
        k_sb = persist.tile([P, NP, N], BF16)
        v_aug = persist.tile([P, NC_, H, HD + 1], BF16)  # pol-scaled + pol col
        outT = persist.tile([P, NP, N], BF16)
        pw_sb = persist.tile([P, CC, C], BF16)
        b_row = persist.tile([1, C], BF16)  # bias as a rank-1 matmul operand
        ones_col = persist.tile([1, P], BF16)
        # bc stationary: row 0 -> out partitions 0:64 (even head), row 32 ->
        # 64:128 (odd head); zeros elsewhere kill the junk rows of rcp2.
        mask33 = persist.tile([33, 2 * HD], BF16)
        polT_sb = persist.tile([P, NC_], F32)
        dmask_sb = persist.tile([P, NC_, P], BF16)
        warm_sb = persist.tile([1, 8], F32)

        wqkp = ctx.enter_context(tc.tile_pool(name="wqkp", bufs=2))
        # E tiles of two blocks stay live at once (av runs one block late)
        Ep = ctx.enter_context(tc.tile_pool(name="Ep", bufs=9))
        rcpp = ctx.enter_context(tc.tile_pool(name="rcpp", bufs=4))
        avsp = ctx.enter_context(tc.tile_pool(name="avsp", bufs=3))

        # PSUM: 8 banks total.  big: 1x[128,1024] (2 banks, filler chains).
        # st: 2x[128,2,512] pair tiles (4 banks, st/exp granularity of two
        # key chunks; also v-chain psum in the prologue and the norm
        # broadcast).  av: 2x[128,512] (2 banks); av results are copied to
        # SBUF right away so the banks never wait on the norm path.
        ps_big = ctx.enter_context(tc.tile_pool(name="ps_big", bufs=1, space="PSUM"))
        ps_st = ctx.enter_context(tc.tile_pool(name="ps_st", bufs=2, space="PSUM"))
        ps_av = ctx.enter_context(tc.tile_pool(name="ps_av", bufs=2, space="PSUM"))

        # xT dies once the last qk chain is emitted (mid-attention), wv after
        # the v chains.  Stack order ... xtp, wvp: wvp closes first, then
        # xtp, then yp opens in the freed space.
        xtp_cm = tc.tile_pool(name="xtp", bufs=1)
        xtp = xtp_cm.__enter__()
        xT_sb = xtp.tile([P, CC, N], BF16, name="xT_sb")
        wvp_cm = tc.tile_pool(name="wvp", bufs=1)
        wvp = wvp_cm.__enter__()
        wv_sb = wvp.tile([P, CC, C], BF16, name="wv_sb")

        # ---------------- prologue DMAs / warmups ------------------------
        # Issue order matters for HBM bandwidth: the first qk chain only
        # needs xT + pair-0 weights, so everything else is issued after
        # them (concurrent bulk DMAs would steal the critical bandwidth).
        nc.gpsimd.memset(warm_sb, 0.0)
        nc.gpsimd.memset(ones_col, 1.0)
        nc.gpsimd.memset(mask33, 0.0)
        nc.gpsimd.memset(mask33[0:1, 0:HD], 1.0)
        nc.gpsimd.memset(mask33[32:33, HD : 2 * HD], 1.0)
        nc.scalar.activation(
            out=warm_sb, in_=warm_sb, func=mybir.ActivationFunctionType.Exp
        )  # pre-load the exp table on ACT

        # first qk link needs only xT[cc0, first half] + wqk[cc0]: those two
        # slices go FIRST on their rings (xT on sync, wqk0 on scalar) so the
        # PE starts as early as possible; the small tensors follow.
        nc.sync.dma_start(out=xT_sb[:, 0, 0:HQ], in_=xT_d[:, 0, 0:HQ])
        wqk_tiles = {}

        def issue_wqk(j, eng, split=False):
            t = wqkp.tile([P, CC, 2, P], BF16, tag="wqk", name=f"wqk{j}")
            if split:
                # the first chain link only needs the cc=0 slice
                eng.dma_start(out=t[:, 0:1], in_=wqk_d[:, 0:1, j, :, :])
                eng.dma_start(out=t[:, 1:4], in_=wqk_d[:, 1:4, j, :, :])
                eng.dma_start(out=t[:, 4:6], in_=wqk_d[:, 4:6, j, :, :])
            else:
                eng.dma_start(out=t, in_=wqk_d[:, :, j, :, :])
            wqk_tiles[j] = t

        issue_wqk(0, nc.scalar, split=True)
        nc.scalar.dma_start(out=polT_sb, in_=polT_d[:])
        nc.sync.dma_start(out=xT_sb[:, 0, HQ:N], in_=xT_d[:, 0, HQ:N])
        # first wv half rides the scalar ring so it lands before the v
        # chains start without delaying the xT stream on the sync ring
        nc.scalar.dma_start(out=wv_sb[:, 0:3], in_=wv_d[:, 0:3, :])
        for cc in range(1, CC):
            nc.sync.dma_start(out=xT_sb[:, cc, :], in_=xT_d[:, cc, :])
        nc.sync.dma_start(out=wv_sb[:, 3:6], in_=wv_d[:, 3:6, :])
        nc.scalar.dma_start(out=dmask_sb, in_=dmask_d[:])
        nc.scalar.dma_start(out=b_row, in_=bias_d[:].unsqueeze(0))
        for nch in range(NC_):
            # pol column of v_aug (bf16): engine copy converts from f32
            nc.gpsimd.tensor_copy(
                out=v_aug[:, nch, :, HD : HD + 1],
                in_=polT_sb[:, nch : nch + 1].unsqueeze(1).broadcast_to((P, H, 1)),
            )

        # ---------------- filler machinery --------------------------------
        # Fillers share ps_big -> strict FIFO.  Protocol: each generator
        # yields a readiness key (or None) BEFORE emitting the next link;
        # advancing it emits exactly one link.  Keys land in `ready` when
        # the producing norm is emitted.
        ready = set()
        fillers = []

        def pull(k):
            done = 0
            while done < k and fillers:
                gen, state = fillers[0]
                if state[0] is not None and state[0] not in ready:
                    break  # head blocked; strict FIFO, stop pulling
                try:
                    state[0] = next(gen)
                except StopIteration:
                    fillers.pop(0)
                done += 1
            return done

        def queue(gen):
            state = [None]
            try:
                state[0] = next(gen)  # prime: allocates tiles, first need
                fillers.append([gen, state])
            except StopIteration:
                pass

        def drain_head_through(predicate):
            """Advance fillers strictly until predicate() or head blocked."""
            while fillers and not predicate():
                if pull(1) == 0:
                    raise RuntimeError("filler deadlock at build time")

        # ---------------- PE work generators -----------------------------
        def mm(scope, *args, **kw):
            with nc.named_scope(scope):
                nc.tensor.matmul(*args, **kw)

        pair_done = set()

        def qk_pair_gen(j):
            # cc-outer, half-inner: both query-half chains advance together
            # per xT channel slice, so the startup DMA stream feeds links as
            # it lands (no half-boundary stall)
            wqk_t = wqk_tiles[j]
            for kk in range(2):
                big = ps_big.tile([P, N], F32, tag="big", name=f"qkb{j}_{kk}")
                for cc in range(CC):
                    for nhh in range(NH):
                        sl = slice(nhh * HQ, (nhh + 1) * HQ)
                        yield None
                        mm(
                            "qk_mm",
                            big[:, sl],
                            lhsT=wqk_t[:, cc, kk, :],
                            rhs=xT_sb[:, cc, sl],
                            start=(cc == 0),
                            stop=(cc == CC - 1),
                        )
                dst = q_sb if kk == 0 else k_sb
                nc.vector.tensor_copy(out=dst[:, j, :], in_=big)
            pair_done.add(j)

        def v_gen(nch):
            # alternate psum between the big pool and an st pair slot so the
            # prologue keeps 3 chain buffers in flight
            if nch % 2 == 0:
                flat = ps_big.tile([P, N], F32, tag="big", name=f"vb{nch}")
            else:
                t = ps_st.tile([P, 2, HQ], F32, tag="st", name=f"vs{nch}")
                flat = t.rearrange("p a b -> p (a b)")
            for o0, osz in ((0, HQ), (HQ, C - HQ)):
                for cc in range(CC):
                    yield None
                    mm(
                        "v_mm",
                        flat[:, o0 : o0 + osz],
                        lhsT=xT_sb[:, cc, nch * P : (nch + 1) * P],
                        rhs=wv_sb[:, cc, o0 : o0 + osz],
                        start=(cc == 0),
                        stop=(cc == CC - 1),
                    )
            nc.vector.tensor_mul(
                out=v_aug[:, nch, :, 0:HD],
                in0=flat[:, 0:C].rearrange("p (h d) -> p h d", d=HD),
                in1=polT_sb[:, nch : nch + 1].unsqueeze(1).broadcast_to((P, H, HD)),
            )

        deferred_y = {}

        def proj_gen(nch, ec_seq, need_nh, skip_ec=None):
            """Proj chains for seq chunk nch, ec-outer so that gated links
            come last and both column sub-chains advance together."""
            big = ps_big.tile([P, N], F32, tag="big", name=f"pb{nch}")
            ecs = [ec for ec in ec_seq if ec != skip_ec]
            spans = ((0, HQ), (HQ, C - HQ))
            for o0, osz in spans:  # rank-1 bias links open both chains
                yield None
                mm(
                    "proj_mm",
                    big[:, o0 : o0 + osz],
                    lhsT=ones_col,
                    rhs=b_row[:, o0 : o0 + osz],
                    start=True,
                    stop=False,
                )
            for i, ec in enumerate(ecs):
                for o0, osz in spans:
                    yield (ec, need_nh)
                    mm(
                        "proj_mm",
                        big[:, o0 : o0 + osz],
                        lhsT=outT[:, ec, nch * P : (nch + 1) * P],
                        rhs=pw_sb[:, ec, o0 : o0 + osz],
                        start=False,
                        stop=(i == len(ecs) - 1),
                    )
            tag, nb = ("yd", 4) if skip_ec is not None else ("y", 1)
            y_t = yp.tile([P, C], F32, tag=tag, bufs=nb, name=f"y{nch}")
            nc.vector.tensor_copy(out=y_t, in_=big[:, 0:C])
            if skip_ec is None:
                yeng = nc.sync if nch % 2 == 0 else nc.scalar
                yeng.dma_start(out=y_d[nch * P : (nch + 1) * P, :], in_=y_t)
            else:
                deferred_y[nch] = y_t

        def finish_proj(nch, ec):
            with nc.named_scope("proj_mm"):
                y_t = deferred_y.pop(nch)
                ps2 = ps_st.tile([P, 2, HQ], F32, tag="st", name=f"fp{nch}")
                nc.tensor.matmul(
                    ps2[:, 0, :],
                    lhsT=outT[:, ec, nch * P : (nch + 1) * P],
                    rhs=pw_sb[:, ec, 0:HQ],
                    start=True,
                    stop=True,
                )
                nc.tensor.matmul(
                    ps2[:, 1, 0 : C - HQ],
                    lhsT=outT[:, ec, nch * P : (nch + 1) * P],
                    rhs=pw_sb[:, ec, HQ:C],
                    start=True,
                    stop=True,
                )
                nc.vector.tensor_add(
                    out=y_t[:, 0:HQ], in0=y_t[:, 0:HQ], in1=ps2[:, 0, :]
                )
                nc.vector.tensor_add(
                    out=y_t[:, HQ:C], in0=y_t[:, HQ:C], in1=ps2[:, 1, 0 : C - HQ]
                )
                yeng = nc.sync if nch % 2 == 0 else nc.scalar
                yeng.dma_start(out=y_d[nch * P : (nch + 1) * P, :], in_=y_t)

        # ---------------- attention block ---------------------------------
        # The PE only sustains its warm streaming rate (~216ns/512-col MM)
        # inside runs of the same chain type; alternating chain types every
        # 1-2 matmuls halves throughput.  So the av matmuls of block b run
        # one block LATE, interleaved with block b+1's st groups, and both
        # come in runs of 4: [st 0,1] [av(prev) 0..3] [st 2,3] [av(prev)
        # 4..7], with fillers pulled in two big chunks per block.
        pair_ctx = {}
        av_queue = None  # deque of previous-block av contexts (set below)

        def drain_av(ctxd, half):
            """Emit av links [4*half, 4*half+4) of a queued block; half==1
            also drains PSUM (avs copy, den row, reciprocal, pair norm)."""
            h2, nh2, E_list = ctxd["h"], ctxd["nh"], ctxd["E"]
            j2, e2 = h2 // 2, h2 % 2
            hp2 = HD * e2
            nsl2 = slice(nh2 * HQ, (nh2 + 1) * HQ)
            if half == 0:
                ctxd["av"] = ps_av.tile([P, HQ], F32, tag="av", name=f"av{h2}_{nh2}")
            av = ctxd["av"]
            for mc in range(4 * half, 4 * half + 4):
                mm(
                    "av_mm",
                    av[0 : HD + 1, :],
                    lhsT=v_aug[:, mc, h2, :],
                    rhs=E_list[mc // 2][:, mc % 2, :],
                    start=(mc == 0),
                    stop=(mc == NC_ - 1),
                )
            if half == 0:
                return
            # Copy av out of PSUM immediately (frees the bank; everything
            # downstream runs off SBUF and can trail without stalling the
            # PE).  Both heads of a pair share one [128,512] avs tile and a
            # den tile with rows 0/32; one DVE reciprocal serves the pair,
            # and at norm time a single masked [33,128] matmul broadcasts
            # both rcp rows into one [128,512] PSUM tile for one DVE
            # multiply per pair.
            if e2 == 0:
                avs = avsp.tile([P, HQ], F32, tag="avs", bufs=3, name=f"avs{h2}_{nh2}")
                den2 = rcpp.tile([33, HQ], F32, tag="den", bufs=2, name=f"dn{h2}_{nh2}")
                pair_ctx["avs"] = avs
                pair_ctx["den2"] = den2
                nc.gpsimd.memset(den2, 1.0)
            else:
                avs = pair_ctx.pop("avs")
                den2 = pair_ctx.pop("den2")
            nc.vector.tensor_copy(out=avs[hp2 : hp2 + HD, :], in_=av[0:HD, :])
            nc.vector.tensor_copy(
                out=den2[32 * e2 : 32 * e2 + 1, :], in_=av[HD : HD + 1, :]
            )
            if e2 == 1:
                # ~18-bit reciprocal at ~1/5 the DVE cost of reciprocal();
                # den >= diag exp > 1e-14 so no edge cases.  The extra copy
                # rounds to f32r for the bc matmul.
                rcp_f = rcpp.tile([33, HQ], F32, tag="rcpf", bufs=2, name=f"rf{h2}_{nh2}")
                nc.vector.reciprocal_approx_fast(out=rcp_f, in_=den2)
                rcp2 = rcpp.tile(
                    [33, HQ], BF16, tag="rcp", bufs=2, name=f"rc{h2}_{nh2}"
                )
                nc.vector.tensor_copy(out=rcp2, in_=rcp_f)

                def norm():
                    with nc.named_scope("norm"):
                        bc = ps_st.tile([P, 2, HQ], F32, tag="st", name=f"bc{h2}_{nh2}")
                        nc.tensor.matmul(
                            bc[:, 0, :],
                            lhsT=mask33,
                            rhs=rcp2,
                            start=True,
                            stop=True,
                        )
                        nc.vector.tensor_mul(
                            out=outT[:, j2, nsl2],
                            in0=avs,
                            in1=bc[:, 0, :],
                        )
                    ready.add((j2, nh2))

                pending.append(norm)

        def att_block(h, nh, pending, budget, last=False):
            """Emit one (head, query-half) block: two st groups of 4 MMs
            (tiles 0,1 then 2,3; one exp per [128,2,512] tile), the queued
            previous block's av groups between them, and fillers in two
            chunks.  Norms pop from `pending` two pairs late so the ~3.3us
            DVE reciprocal never gates the PE stream."""
            j, e = h // 2, h % 2
            hp = HD * e
            nsl = slice(nh * HQ, (nh + 1) * HQ)
            E_list = []
            spent = [0]

            def pace(frac):
                want = int(budget * frac + 0.5)
                if want > spent[0]:
                    spent[0] += pull(want - spent[0])

            for half in range(2):
                for t in (2 * half, 2 * half + 1):
                    st_t = ps_st.tile(
                        [P, 2, HQ], F32, tag="st", name=f"st{h}_{nh}_{t}"
                    )
                    for k in range(2):
                        mc = 2 * t + k
                        mm(
                            "st_mm",
                            st_t[:, k, :],
                            lhsT=k_sb[hp : hp + HD, j, mc * P : (mc + 1) * P],
                            rhs=q_sb[hp : hp + HD, j, nsl],
                            start=True,
                            stop=True,
                        )
                    E_t = Ep.tile([P, 2, HQ], BF16, tag="E", name=f"E{h}_{nh}_{t}")
                    nc.scalar.activation(
                        out=E_t,
                        in_=st_t,
                        func=mybir.ActivationFunctionType.Exp,
                        scale=SCALE,
                    )
                    if t in (2 * nh, 2 * nh + 1):
                        # both chunks of this pair hold diagonal blocks; one
                        # strided multiply covers them
                        diag_ap = bass.AP(
                            tensor=E_t.tensor,
                            offset=E_t.offset + (2 * t * P - nh * HQ),
                            ap=[E_t.ap[0], [HQ + P, 2], [1, P]],
                        )
                        nc.gpsimd.tensor_mul(
                            out=diag_ap,
                            in0=diag_ap,
                            in1=dmask_sb[:, 2 * t : 2 * t + 2, :],
                        )
                    E_list.append(E_t)
                if av_queue:
                    drain_av(av_queue[0], half)
                    if half == 1:
                        av_queue.popleft()
                if half == 0 and len(pending) >= 2:
                    pending.popleft()()
                pace(0.5 * (half + 1))
            ctx_own = dict(h=h, nh=nh, E=E_list)
            if last:
                # final block: no next block to carry its av, emit inline so
                # the last norms land before the tail
                for half2 in range(2):
                    drain_av(ctx_own, half2)
            else:
                av_queue.append(ctx_own)

        # ---------------- master schedule ----------------------------------
        # prologue PE work: qk pairs 0-2 FIRST (their weights land long
        # before wv does), then the v chains — the qk work covers the wv
        # transfer window instead of the PE idling on it.
        queue(qk_pair_gen(0))
        drain_head_through(lambda: 0 in pair_done)
        issue_wqk(1, nc.scalar)
        for nch in range(NC_):
            queue(v_gen(nch))
        queue(qk_pair_gen(1))
        issue_wqk(2, nc.scalar)
        queue(qk_pair_gen(2))
        while fillers:
            if pull(8) == 0:
                raise RuntimeError("v-phase deadlock")

        wvp_cm.__exit__(None, None, None)  # free wv SBUF
        # proj weights are only needed from the second query-half on;
        # issuing here keeps them off the startup-critical HBM window.
        nc.sync.dma_start(out=pw_sb, in_=pw_d[:])
        yp = None  # created mid-attention, after xtp closes

        order1 = [10, 11] + list(range(10))  # pair 4 (h=8,9) normalized last
        blocks = [(h, 0) for h in range(H)] + [(h, 1) for h in order1]
        # nh=1 proj link order must follow order1 norm completion
        ec_seq1 = [5, 0, 1, 2, 3, 4]

        from collections import deque

        pending = deque()
        av_queue = deque()
        for bi, (h, nh) in enumerate(blocks):
            if nh == 0:
                if bi % 2 == 0 and 3 + bi // 2 < NP:
                    jn = 3 + bi // 2  # pairs 3..5 queued at blocks 0,2,4
                    issue_wqk(jn, nc.sync)
                    queue(qk_pair_gen(jn))
                if h % 2 == 0 and h // 2 >= 3:
                    jneed = h // 2
                    drain_head_through(lambda: jneed in pair_done)
                if bi == 10:
                    # all qk chains emitted -> xT dead; reuse for y staging
                    xtp_cm.__exit__(None, None, None)
                    yp = ctx.enter_context(tc.tile_pool(name="yp", bufs=2))
                budget = 8 if bi < 10 else 3
            else:
                if bi == H:
                    # proj for the first query half: pairs 0..4 are normed
                    # by now (delay-3 pops), pair 5 unlocks at bi==H+2
                    for nch in range(4):
                        queue(proj_gen(nch, list(range(CC)), 0))
                elif bi == H + 2:
                    for nch in range(4, NC_):
                        queue(proj_gen(nch, ec_seq1, 1, skip_ec=4))
                budget = 8 if bi <= H + 1 else (9 if bi < H + 6 else 10)
            att_block(h, nh, pending, budget, last=(bi == len(blocks) - 1))

        # tail: drain the last block's av, flush remaining norms (they
        # unlock the last gated proj links), drain leftovers, then the
        # deferred pair-4 links
        while av_queue:
            drain_av(av_queue[0], 0)
            drain_av(av_queue[0], 1)
            av_queue.popleft()
        while pending:
            pending.popleft()()
        while fillers:
            if pull(100) == 0:
                raise RuntimeError("tail filler deadlock")
        for nch in range(4, NC_):
            finish_proj(nch, ec=4)

    nc.compile()
    return nc


_NC_CACHE = None


def _get_nc():
    global _NC_CACHE
    if _NC_CACHE is None:
        _NC_CACHE = _build_nc()
    return _NC_CACHE


def _pack_inputs(x, policy, qkv_w, proj_w, proj_b):
    """Host-side packing into partition-major DRAM layouts (see _build_nc).
    All matmul operands ship as bf16: halves the startup HBM traffic and
    enables FWL weight loads; adds ~2e-3 rel err (gate is 2e-2)."""
    bf = ml_dtypes.bfloat16
    wqkT = qkv_w[: 2 * H * HD].T  # [768, 1536]
    wqk = np.ascontiguousarray(
        wqkT.reshape(CC, P, 2, NP, P).transpose(1, 0, 3, 2, 4)
    ).astype(bf)  # [p, cc, j, kk, d2]
    wv = np.ascontiguousarray(
        qkv_w[2 * H * HD :].T.reshape(CC, P, C).transpose(1, 0, 2)
    ).astype(bf)
    pw = np.ascontiguousarray(proj_w.T.reshape(CC, P, C).transpose(1, 0, 2)).astype(bf)

    in_maps = []
    for b in range(B):
        xT = np.ascontiguousarray(
            x[b].T.reshape(CC, P, N).transpose(1, 0, 2)
        ).astype(bf)
        pol = policy[b, :, 0]
        polc = np.maximum(pol, 1e-30)
        polT = np.ascontiguousarray(pol.reshape(NC_, P).T)
        dmask = np.ones((P, NC_, P), dtype=np.float32)
        rng = np.arange(P)
        for kch in range(NC_):
            dmask[rng, kch, rng] = 1.0 / polc[kch * P + rng]
        in_maps.append(
            dict(
                xT=xT,
                wqk=wqk,
                wv=wv,
                pw=pw,
                bias=proj_b.astype(bf),
                polT=polT.astype(np.float32),
                dmask=dmask.astype(bf),
            )
        )
    return in_maps


def kernel(x, policy, qkv_w, proj_w, proj_b):
    global LAST_RESULTS
    x = np.asarray(x, dtype=np.float32)
    policy = np.asarray(policy, dtype=np.float32)
    qkv_w = np.asarray(qkv_w, dtype=np.float32)
    proj_w = np.asarray(proj_w, dtype=np.float32)
    proj_b = np.asarray(proj_b, dtype=np.float32)

    in_maps = _pack_inputs(x, policy, qkv_w, proj_w, proj_b)

    nc = _get_nc()
    trace = os.environ.get("KERNEL_TRACE", "0") == "1"
    res = run_bass_kernel_spmd(
        nc,
        in_maps,
        core_ids=list(range(B)),
        trace=trace,
        trace_cores=list(range(B)) if trace else None,
        stitch_traces=False,
    )
    LAST_RESULTS = res
    return np.stack([res.results[b]["y"] for b in range(B)], axis=0)



# revision 44
# speedup vs baseline: 1.0078x; 1.0078x over previous
"""Trainium2 Bass kernel for policy-weighted multi-head attention.

Reference computation (per batch b, 8 batches):
    qkv = x @ qkv_w.T                     # [N, 3*H*HD]
    q, k, v per head                      # H=12 heads, HD=64
    s = (q * HD^-0.5) @ k.T               # [N, N]
    a[n,m] ~ exp(s[n,m]) * (pol[m] + (1-pol[m])*eye)  normalized over m
    out = a @ v ; y = out @ proj_w.T + b

Sharding: pure data parallel, one batch per NeuronCore (8 cores).

Kernel strategy (per core).  The governing constraint is that every
engine queue is in-order: a matmul whose semaphore is not yet satisfied
blocks everything queued behind it, and each such stall costs the PE
~200ns of pipeline refill on top of the wait.  The schedule therefore
(a) keeps every producer (exp on ACT, reciprocal/norm on DVE) well
ahead of its PE consumer and (b) emits PE work in same-chain runs of
4-8 matmuls (chain-type switches cost ~150-200ns each):
  - Everything the PE multiplies is bf16 (shipped bf16 from the host:
    halves startup HBM traffic, enables fast weight loads, 2x DVE on
    the PSUM->SBUF casts).  Accumulation stays f32 in PSUM; measured
    rel err 5.7e-3 vs the 2e-2 gate.
  - Host pre-packs x/weights into partition-major layouts so every SBUF
    load is one DMA descriptor; no on-chip transposes.  The first qk
    chain's two slices go first on separate DMA rings (xT on sync,
    wqk0 on scalar); qk chains run cc-outer/half-inner so links start
    as their xT slice lands.
  - Attention in the S^T layout (partitions = key index m): the softmax
    denominator folds into the a@v matmul via a pol column appended to V
    (V rows pre-scaled by pol[m]); the diagonal policy term is a masked
    multiply with a precomputed [128, 8, 128] mask (diag = 1/pol).
  - st/exp run at [128,2,512] pair granularity (one exp per 1024 cols).
    q and k live in separate bf16 tiles (distinct SBUF regions for the
    moving and LDWEIGHTS fetch streams).
  - The a@v matmuls of block b run one block LATE, in two runs of 4
    interleaved with block b+1's two st groups of 4: by then all E
    tiles exist, so av links are never exp-gated and pad the window
    until exp(t-1) frees the next st PSUM buf (2-buf rotation).  The
    last block drains its own av inline.  E pool holds 9 tiles (two
    blocks in flight).
  - Normalization per head PAIR, off the critical path: av is copied to
    SBUF at once (releasing its PSUM bank); the denominator rows land
    on partitions 0/32 of a pair tile via ACT Copy; reciprocal_approx_
    fast (~1/5 the DVE cost of reciprocal(), 18 bits) serves both
    heads; at norm time one masked-[33,128] matmul broadcasts both rcp
    rows into a [128,512] PSUM tile for one DVE multiply per pair.
    Norms pop from a pending queue ~2 pairs late.
  - Query-half-outer schedule: heads 0..11 on queries 0:512, then on
    512:1024 (rotated so pair 4 finishes last).  PE filler work rides in
    the holes the st->exp->av pipeline would otherwise leave: qk chains
    for pairs 3..5 fill half 0 (pairs 0..2 load in the prologue); the
    output projection fills half 1, ec-outer so norm-gated links come
    last, with bias folded in as a rank-1 first link and only pair 4's
    links deferred to the tail.  Fillers are strict-FIFO (they share
    PSUM) and readiness-gated so no instruction precedes its producer.
  - exp on Scalar (table pre-warmed), diag multiplies on GpSimd, PSUM
    drains (qk casts, av/y copies) and normalization on DVE; y DMAs
    alternate rings so tail completions overlap.
  - max-subtraction and the eps terms of the softmax are dropped: logits
    are ~N(0,1) so exp() cannot overflow; the eps corrections are ~1e-9.
"""

import os

os.environ.setdefault("JAX_PLATFORMS", "axon")

from contextlib import ExitStack

import ml_dtypes
import numpy as np

import concourse.bass as bass
import concourse.tile as tile
from concourse import bacc, mybir
from concourse.bass_utils import run_bass_kernel_spmd

B, N, C = 8, 1024, 768
H, HD = 12, 64
SCALE = HD ** (-0.5)
F32 = mybir.dt.float32
F32R = mybir.dt.float32r
BF16 = mybir.dt.bfloat16
P = 128
NC_ = N // P  # 8 seq chunks
CC = C // P  # 6 channel chunks / head pairs
NP = H // 2  # 6 head pairs
NH = 2  # query halves
HQ = N // NH  # 512

LAST_RESULTS = None  # BassKernelResults of the most recent run (for test.py)


def _build_nc():
    nc = bacc.Bacc(None, target_bir_lowering=False)

    xT_d = nc.dram_tensor("xT", [P, CC, N], BF16, kind="ExternalInput")
    wqk_d = nc.dram_tensor("wqk", [P, CC, NP, 2, P], BF16, kind="ExternalInput")
    wv_d = nc.dram_tensor("wv", [P, CC, C], BF16, kind="ExternalInput")
    pw_d = nc.dram_tensor("pw", [P, CC, C], BF16, kind="ExternalInput")
    bias_d = nc.dram_tensor("bias", [C], BF16, kind="ExternalInput")
    polT_d = nc.dram_tensor("polT", [P, NC_], F32, kind="ExternalInput")
    dmask_d = nc.dram_tensor("dmask", [P, NC_, P], BF16, kind="ExternalInput")
    y_d = nc.dram_tensor("y", [N, C], F32, kind="ExternalOutput")

    with ExitStack() as ctx:
        tc = ctx.enter_context(tile.TileContext(nc))

        persist = ctx.enter_context(tc.tile_pool(name="persist", bufs=1))
        # q and k in bf16, in SEPARATE tiles: halves the DVE cast cost and
        # SBUF footprint, enables FWL on the st stationary (128 cols, non-
        # fp32), and keeps the st moving + LDWEIGHTS fetch streams out of
        # the same SBUF region.
        q_sb = persist.tile([P, NP, N], BF16)
        k_sb = persist.tile([P, NP, N], BF16)
        v_aug = persist.tile([P, NC_, H, HD + 1], BF16)  # pol-scaled + pol col
        outT = persist.tile([P, NP, N], BF16)
        pw_sb = persist.tile([P, CC, C], BF16)
        b_row = persist.tile([1, C], BF16)  # bias as a rank-1 matmul operand
        ones_col = persist.tile([1, P], BF16)
        # bc stationary: row 0 -> out partitions 0:64 (even head), row 32 ->
        # 64:128 (odd head); zeros elsewhere kill the junk rows of rcp2.
        mask33 = persist.tile([33, 2 * HD], BF16)
        polT_sb = persist.tile([P, NC_], F32)
        dmask_sb = persist.tile([P, NC_, P], BF16)
        warm_sb = persist.tile([1, 8], F32)

        wqkp = ctx.enter_context(tc.tile_pool(name="wqkp", bufs=2))
        # E tiles of two blocks stay live at once (av runs one block late)
        Ep = ctx.enter_context(tc.tile_pool(name="Ep", bufs=9))
        rcpp = ctx.enter_context(tc.tile_pool(name="rcpp", bufs=4))
        avsp = ctx.enter_context(tc.tile_pool(name="avsp", bufs=3))

        # PSUM: 8 banks total.  big: 1x[128,1024] (2 banks, filler chains).
        # st: 2x[128,2,512] pair tiles (4 banks, st/exp granularity of two
        # key chunks; also v-chain psum in the prologue and the norm
        # broadcast).  av: 2x[128,512] (2 banks); av results are copied to
        # SBUF right away so the banks never wait on the norm path.
        ps_big = ctx.enter_context(tc.tile_pool(name="ps_big", bufs=1, space="PSUM"))
        ps_st = ctx.enter_context(tc.tile_pool(name="ps_st", bufs=2, space="PSUM"))
        ps_av = ctx.enter_context(tc.tile_pool(name="ps_av", bufs=2, space="PSUM"))

        # xT dies once the last qk chain is emitted (mid-attention), wv after
        # the v chains.  Stack order ... xtp, wvp: wvp closes first, then
        # xtp, then yp opens in the freed space.
        xtp_cm = tc.tile_pool(name="xtp", bufs=1)
        xtp = xtp_cm.__enter__()
        xT_sb = xtp.tile([P, CC, N], BF16, name="xT_sb")
        wvp_cm = tc.tile_pool(name="wvp", bufs=1)
        wvp = wvp_cm.__enter__()
        wv_sb = wvp.tile([P, CC, C], BF16, name="wv_sb")

        # ---------------- prologue DMAs / warmups ------------------------
        # Issue order matters for HBM bandwidth: the first qk chain only
        # needs xT + pair-0 weights, so everything else is issued after
        # them (concurrent bulk DMAs would steal the critical bandwidth).
        nc.gpsimd.memset(warm_sb, 0.0)
        nc.gpsimd.memset(ones_col, 1.0)
        nc.gpsimd.memset(mask33, 0.0)
        nc.gpsimd.memset(mask33[0:1, 0:HD], 1.0)
        nc.gpsimd.memset(mask33[32:33, HD : 2 * HD], 1.0)
        nc.scalar.activation(
            out=warm_sb, in_=warm_sb, func=mybir.ActivationFunctionType.Exp
        )  # pre-load the exp table on ACT

        # first qk link needs only xT[cc0, first half] + wqk[cc0]: those two
        # slices go FIRST on their rings (xT on sync, wqk0 on scalar) so the
        # PE starts as early as possible; the small tensors follow.
        nc.sync.dma_start(out=xT_sb[:, 0, 0:HQ], in_=xT_d[:, 0, 0:HQ])
        wqk_tiles = {}

        def issue_wqk(j, eng, split=False):
            t = wqkp.tile([P, CC, 2, P], BF16, tag="wqk", name=f"wqk{j}")
            if split:
                # the first chain link only needs the cc=0 slice
                eng.dma_start(out=t[:, 0:1], in_=wqk_d[:, 0:1, j, :, :])
                eng.dma_start(out=t[:, 1:4], in_=wqk_d[:, 1:4, j, :, :])
                eng.dma_start(out=t[:, 4:6], in_=wqk_d[:, 4:6, j, :, :])
            else:
                eng.dma_start(out=t, in_=wqk_d[:, :, j, :, :])
            wqk_tiles[j] = t

        issue_wqk(0, nc.scalar, split=True)
        nc.scalar.dma_start(out=polT_sb, in_=polT_d[:])
        nc.sync.dma_start(out=xT_sb[:, 0, HQ:N], in_=xT_d[:, 0, HQ:N])
        # first wv half rides the scalar ring so it lands before the v
        # chains start without delaying the xT stream on the sync ring
        nc.scalar.dma_start(out=wv_sb[:, 0:3], in_=wv_d[:, 0:3, :])
        for cc in range(1, CC):
            nc.sync.dma_start(out=xT_sb[:, cc, :], in_=xT_d[:, cc, :])
        nc.sync.dma_start(out=wv_sb[:, 3:6], in_=wv_d[:, 3:6, :])
        nc.scalar.dma_start(out=dmask_sb, in_=dmask_d[:])
        nc.scalar.dma_start(out=b_row, in_=bias_d[:].unsqueeze(0))
        for nch in range(NC_):
            # pol column of v_aug (bf16): engine copy converts from f32
            nc.gpsimd.tensor_copy(
                out=v_aug[:, nch, :, HD : HD + 1],
                in_=polT_sb[:, nch : nch + 1].unsqueeze(1).broadcast_to((P, H, 1)),
            )

        # ---------------- filler machinery --------------------------------
        # Fillers share ps_big -> strict FIFO.  Protocol: each generator
        # yields a readiness key (or None) BEFORE emitting the next link;
        # advancing it emits exactly one link.  Keys land in `ready` when
        # the producing norm is emitted.
        ready = set()
        fillers = []

        def pull(k):
            done = 0
            while done < k and fillers:
                gen, state = fillers[0]
                if state[0] is not None and state[0] not in ready:
                    break  # head blocked; strict FIFO, stop pulling
                try:
                    state[0] = next(gen)
                except StopIteration:
                    fillers.pop(0)
                done += 1
            return done

        def queue(gen):
            state = [None]
            try:
                state[0] = next(gen)  # prime: allocates tiles, first need
                fillers.append([gen, state])
            except StopIteration:
                pass

        def drain_head_through(predicate):
            """Advance fillers strictly until predicate() or head blocked."""
            while fillers and not predicate():
                if pull(1) == 0:
                    raise RuntimeError("filler deadlock at build time")

        # ---------------- PE work generators -----------------------------
        def mm(scope, *args, **kw):
            with nc.named_scope(scope):
                nc.tensor.matmul(*args, **kw)

        pair_done = set()

        def qk_pair_gen(j):
            # cc-outer, half-inner: both query-half chains advance together
            # per xT channel slice, so the startup DMA stream feeds links as
            # it lands (no half-boundary stall)
            wqk_t = wqk_tiles[j]
            for kk in range(2):
                big = ps_big.tile([P, N], F32, tag="big", name=f"qkb{j}_{kk}")
                for cc in range(CC):
                    for nhh in range(NH):
                        sl = slice(nhh * HQ, (nhh + 1) * HQ)
                        yield None
                        mm(
                            "qk_mm",
                            big[:, sl],
                            lhsT=wqk_t[:, cc, kk, :],
                            rhs=xT_sb[:, cc, sl],
                            start=(cc == 0),
                            stop=(cc == CC - 1),
                        )
                dst = q_sb if kk == 0 else k_sb
                nc.vector.tensor_copy(out=dst[:, j, :], in_=big)
            pair_done.add(j)

        def v_gen(nch):
            # alternate psum between the big pool and an st pair slot so the
            # prologue keeps 3 chain buffers in flight
            if nch % 2 == 0:
                flat = ps_big.tile([P, N], F32, tag="big", name=f"vb{nch}")
            else:
                t = ps_st.tile([P, 2, HQ], F32, tag="st", name=f"vs{nch}")
                flat = t.rearrange("p a b -> p (a b)")
            for o0, osz in ((0, HQ), (HQ, C - HQ)):
                for cc in range(CC):
                    yield None
                    mm(
                        "v_mm",
                        flat[:, o0 : o0 + osz],
                        lhsT=xT_sb[:, cc, nch * P : (nch + 1) * P],
                        rhs=wv_sb[:, cc, o0 : o0 + osz],
                        start=(cc == 0),
                        stop=(cc == CC - 1),
                    )
            nc.vector.tensor_mul(
                out=v_aug[:, nch, :, 0:HD],
                in0=flat[:, 0:C].rearrange("p (h d) -> p h d", d=HD),
                in1=polT_sb[:, nch : nch + 1].unsqueeze(1).broadcast_to((P, H, HD)),
            )

        deferred_y = {}

        def proj_gen(nch, ec_seq, need_nh, skip_ec=None):
            """Proj chains for seq chunk nch, ec-outer so that gated links
            come last and both column sub-chains advance together."""
            big = ps_big.tile([P, N], F32, tag="big", name=f"pb{nch}")
            ecs = [ec for ec in ec_seq if ec != skip_ec]
            spans = ((0, HQ), (HQ, C - HQ))
            for o0, osz in spans:  # rank-1 bias links open both chains
                yield None
                mm(
                    "proj_mm",
                    big[:, o0 : o0 + osz],
                    lhsT=ones_col,
                    rhs=b_row[:, o0 : o0 + osz],
                    start=True,
                    stop=False,
                )
            for i, ec in enumerate(ecs):
                for o0, osz in spans:
                    yield (ec, need_nh)
                    mm(
                        "proj_mm",
                        big[:, o0 : o0 + osz],
                        lhsT=outT[:, ec, nch * P : (nch + 1) * P],
                        rhs=pw_sb[:, ec, o0 : o0 + osz],
                        start=False,
                        stop=(i == len(ecs) - 1),
                    )
            tag, nb = ("yd", 4) if skip_ec is not None else ("y", 1)
            y_t = yp.tile([P, C], F32, tag=tag, bufs=nb, name=f"y{nch}")
            nc.vector.tensor_copy(out=y_t, in_=big[:, 0:C])
            if skip_ec is None:
                yeng = nc.sync if nch % 2 == 0 else nc.scalar
                yeng.dma_start(out=y_d[nch * P : (nch + 1) * P, :], in_=y_t)
            else:
                deferred_y[nch] = y_t

        def finish_proj(nch, ec):
            with nc.named_scope("proj_mm"):
                y_t = deferred_y.pop(nch)
                ps2 = ps_st.tile([P, 2, HQ], F32, tag="st", name=f"fp{nch}")
                nc.tensor.matmul(
                    ps2[:, 0, :],
                    lhsT=outT[:, ec, nch * P : (nch + 1) * P],
                    rhs=pw_sb[:, ec, 0:HQ],
                    start=True,
                    stop=True,
                )
                nc.tensor.matmul(
                    ps2[:, 1, 0 : C - HQ],
                    lhsT=outT[:, ec, nch * P : (nch + 1) * P],
                    rhs=pw_sb[:, ec, HQ:C],
                    start=True,
                    stop=True,
                )
                nc.vector.tensor_add(
                    out=y_t[:, 0:HQ], in0=y_t[:, 0:HQ], in1=ps2[:, 0, :]
                )
                nc.vector.tensor_add(
                    out=y_t[:, HQ:C], in0=y_t[:, HQ:C], in1=ps2[:, 1, 0 : C - HQ]
                )
                yeng = nc.sync if nch % 2 == 0 else nc.scalar
                yeng.dma_start(out=y_d[nch * P : (nch + 1) * P, :], in_=y_t)

        # ---------------- attention block ---------------------------------
        # The PE only sustains its warm streaming rate (~216ns/512-col MM)
        # inside runs of the same chain type; alternating chain types every
        # 1-2 matmuls halves throughput.  So the av matmuls of block b run
        # one block LATE, interleaved with block b+1's st groups, and both
        # come in runs of 4: [st 0,1] [av(prev) 0..3] [st 2,3] [av(prev)
        # 4..7], with fillers pulled in two big chunks per block.
        pair_ctx = {}
        av_queue = None  # deque of previous-block av contexts (set below)

        def drain_av(ctxd, half):
            """Emit av links [4*half, 4*half+4) of a queued block; half==1
            also drains PSUM (avs copy, den row, reciprocal, pair norm)."""
            h2, nh2, E_list = ctxd["h"], ctxd["nh"], ctxd["E"]
            j2, e2 = h2 // 2, h2 % 2
            hp2 = HD * e2
            nsl2 = slice(nh2 * HQ, (nh2 + 1) * HQ)
            if half == 0:
                ctxd["av"] = ps_av.tile([P, HQ], F32, tag="av", name=f"av{h2}_{nh2}")
            av = ctxd["av"]
            for mc in range(4 * half, 4 * half + 4):
                mm(
                    "av_mm",
                    av[0 : HD + 1, :],
                    lhsT=v_aug[:, mc, h2, :],
                    rhs=E_list[mc // 2][:, mc % 2, :],
                    start=(mc == 0),
                    stop=(mc == NC_ - 1),
                )
            if half == 0:
                return
            # Copy av out of PSUM immediately (frees the bank; everything
            # downstream runs off SBUF and can trail without stalling the
            # PE).  Both heads of a pair share one [128,512] avs tile and a
            # den tile with rows 0/32; one DVE reciprocal serves the pair,
            # and at norm time a single masked [33,128] matmul broadcasts
            # both rcp rows into one [128,512] PSUM tile for one DVE
            # multiply per pair.
            if e2 == 0:
                avs = avsp.tile([P, HQ], F32, tag="avs", bufs=3, name=f"avs{h2}_{nh2}")
                den2 = rcpp.tile([33, HQ], F32, tag="den", bufs=2, name=f"dn{h2}_{nh2}")
                pair_ctx["avs"] = avs
                pair_ctx["den2"] = den2
                nc.gpsimd.memset(den2, 1.0)
            else:
                avs = pair_ctx.pop("avs")
                den2 = pair_ctx.pop("den2")
            nc.vector.tensor_copy(out=avs[hp2 : hp2 + HD, :], in_=av[0:HD, :])
            nc.vector.tensor_copy(
                out=den2[32 * e2 : 32 * e2 + 1, :], in_=av[HD : HD + 1, :]
            )
            if e2 == 1:
                # ~18-bit reciprocal at ~1/5 the DVE cost of reciprocal();
                # den >= diag exp > 1e-14 so no edge cases.  The extra copy
                # rounds to f32r for the bc matmul.
                rcp_f = rcpp.tile([33, HQ], F32, tag="rcpf", bufs=2, name=f"rf{h2}_{nh2}")
                nc.vector.reciprocal_approx_fast(out=rcp_f, in_=den2)
                rcp2 = rcpp.tile(
                    [33, HQ], BF16, tag="rcp", bufs=2, name=f"rc{h2}_{nh2}"
                )
                nc.vector.tensor_copy(out=rcp2, in_=rcp_f)

                def norm():
                    with nc.named_scope("norm"):
                        bc = ps_st.tile([P, 2, HQ], F32, tag="st", name=f"bc{h2}_{nh2}")
                        nc.tensor.matmul(
                            bc[:, 0, :],
                            lhsT=mask33,
                            rhs=rcp2,
                            start=True,
                            stop=True,
                        )
                        nc.vector.tensor_mul(
                            out=outT[:, j2, nsl2],
                            in0=avs,
                            in1=bc[:, 0, :],
                        )
                    ready.add((j2, nh2))

                pending.append(norm)

        def att_block(h, nh, pending, budget, last=False):
            """Emit one (head, query-half) block: two st groups of 4 MMs
            (tiles 0,1 then 2,3; one exp per [128,2,512] tile), the queued
            previous block's av groups between them, and fillers in two
            chunks.  Norms pop from `pending` two pairs late so the ~3.3us
            DVE reciprocal never gates the PE stream."""
            j, e = h // 2, h % 2
            hp = HD * e
            nsl = slice(nh * HQ, (nh + 1) * HQ)
            E_list = []
            spent = [0]

            def pace(frac):
                want = int(budget * frac + 0.5)
                if want > spent[0]:
                    spent[0] += pull(want - spent[0])

            for half in range(2):
                for t in (2 * half, 2 * half + 1):
                    st_t = ps_st.tile(
                        [P, 2, HQ], F32, tag="st", name=f"st{h}_{nh}_{t}"
                    )
                    for k in range(2):
                        mc = 2 * t + k
                        mm(
                            "st_mm",
                            st_t[:, k, :],
                            lhsT=k_sb[hp : hp + HD, j, mc * P : (mc + 1) * P],
                            rhs=q_sb[hp : hp + HD, j, nsl],
                            start=True,
                            stop=True,
                        )
                    E_t = Ep.tile([P, 2, HQ], BF16, tag="E", name=f"E{h}_{nh}_{t}")
                    nc.scalar.activation(
                        out=E_t,
                        in_=st_t,
                        func=mybir.ActivationFunctionType.Exp,
                        scale=SCALE,
                    )
                    if t in (2 * nh, 2 * nh + 1):
                        # both chunks of this pair hold diagonal blocks; one
                        # strided multiply covers them
                        diag_ap = bass.AP(
                            tensor=E_t.tensor,
                            offset=E_t.offset + (2 * t * P - nh * HQ),
                            ap=[E_t.ap[0], [HQ + P, 2], [1, P]],
                        )
                        nc.gpsimd.tensor_mul(
                            out=diag_ap,
                            in0=diag_ap,
                            in1=dmask_sb[:, 2 * t : 2 * t + 2, :],
                        )
                    E_list.append(E_t)
                if av_queue:
                    drain_av(av_queue[0], half)
                    if half == 1:
                        av_queue.popleft()
                if half == 0 and len(pending) >= 2:
                    pending.popleft()()
                pace(0.5 * (half + 1))
            ctx_own = dict(h=h, nh=nh, E=E_list)
            if last:
                # final block: no next block to carry its av, emit inline so
                # the last norms land before the tail
                for half2 in range(2):
                    drain_av(ctx_own, half2)
            else:
                av_queue.append(ctx_own)

        # ---------------- master schedule ----------------------------------
        # prologue PE work: qk pairs 0-2 FIRST (their weights land long
        # before wv does), then the v chains — the qk work covers the wv
        # transfer window instead of the PE idling on it.
        queue(qk_pair_gen(0))
        drain_head_through(lambda: 0 in pair_done)
        issue_wqk(1, nc.scalar)
        for nch in range(NC_):
            queue(v_gen(nch))
        queue(qk_pair_gen(1))
        issue_wqk(2, nc.scalar)
        queue(qk_pair_gen(2))
        while fillers:
            if pull(8) == 0:
                raise RuntimeError("v-phase deadlock")

        wvp_cm.__exit__(None, None, None)  # free wv SBUF
        # proj weights are only needed from the second query-half on;
        # issuing here keeps them off the startup-critical HBM window.
        nc.sync.dma_start(out=pw_sb, in_=pw_d[:])
        yp = None  # created mid-attention, after xtp closes

        order1 = [10, 11] + list(range(10))  # pair 4 (h=8,9) normalized last
        blocks = [(h, 0) for h in range(H)] + [(h, 1) for h in order1]
        # nh=1 proj link order must follow order1 norm completion
        ec_seq1 = [5, 0, 1, 2, 3, 4]

        from collections import deque

        pending = deque()
        av_queue = deque()
        for bi, (h, nh) in enumerate(blocks):
            if nh == 0:
                if bi % 2 == 0 and 3 + bi // 2 < NP:
                    jn = 3 + bi // 2  # pairs 3..5 queued at blocks 0,2,4
                    issue_wqk(jn, nc.sync)
                    queue(qk_pair_gen(jn))
                if h % 2 == 0 and h // 2 >= 3:
                    jneed = h // 2
                    drain_head_through(lambda: jneed in pair_done)
                if bi == 10:
                    # all qk chains emitted -> xT dead; reuse for y staging
                    xtp_cm.__exit__(None, None, None)
                    yp = ctx.enter_context(tc.tile_pool(name="yp", bufs=2))
                budget = 8 if bi < 10 else 3
            else:
                if bi == H:
                    # proj for the first query half: pairs 0..4 are normed
                    # by now (delay-3 pops), pair 5 unlocks at bi==H+2
                    for nch in range(4):
                        queue(proj_gen(nch, list(range(CC)), 0))
                elif bi == H + 2:
                    for nch in range(4, NC_):
                        queue(proj_gen(nch, ec_seq1, 1, skip_ec=4))
                budget = 8 if bi <= H + 1 else (9 if bi < H + 6 else 10)
            att_block(h, nh, pending, budget, last=(bi == len(blocks) - 1))

        # tail: drain the last block's av, flush remaining norms (they
        # unlock the last gated proj links), drain leftovers, then the
        # deferred pair-4 links
        while av_queue:
            drain_av(av_queue[0], 0)
            drain_av(av_queue[0], 1)
            av_queue.popleft()
        while pending:
            pending.popleft()()
        while fillers:
            if pull(100) == 0:
                raise RuntimeError("tail filler deadlock")
        for nch in range(4, NC_):
            finish_proj(nch, ec=4)

    nc.compile()
    return nc


_NC_CACHE = None


def _get_nc():
    global _NC_CACHE
    if _NC_CACHE is None:
        _NC_CACHE = _build_nc()
    return _NC_CACHE


def _pack_inputs(x, policy, qkv_w, proj_w, proj_b):
    """Host-side packing into partition-major DRAM layouts (see _build_nc).
    All matmul operands ship as bf16: halves the startup HBM traffic and
    enables FWL weight loads; adds ~2e-3 rel err (gate is 2e-2)."""
    bf = ml_dtypes.bfloat16
    wqkT = qkv_w[: 2 * H * HD].T  # [768, 1536]
    wqk = np.ascontiguousarray(
        wqkT.reshape(CC, P, 2, NP, P).transpose(1, 0, 3, 2, 4)
    ).astype(bf)  # [p, cc, j, kk, d2]
    wv = np.ascontiguousarray(
        qkv_w[2 * H * HD :].T.reshape(CC, P, C).transpose(1, 0, 2)
    ).astype(bf)
    pw = np.ascontiguousarray(proj_w.T.reshape(CC, P, C).transpose(1, 0, 2)).astype(bf)

    in_maps = []
    for b in range(B):
        xT = np.ascontiguousarray(
            x[b].T.reshape(CC, P, N).transpose(1, 0, 2)
        ).astype(bf)
        pol = policy[b, :, 0]
        polc = np.maximum(pol, 1e-30)
        polT = np.ascontiguousarray(pol.reshape(NC_, P).T)
        dmask = np.ones((P, NC_, P), dtype=np.float32)
        rng = np.arange(P)
        for kch in range(NC_):
            dmask[rng, kch, rng] = 1.0 / polc[kch * P + rng]
        in_maps.append(
            dict(
                xT=xT,
                wqk=wqk,
                wv=wv,
                pw=pw,
                bias=proj_b.astype(bf),
                polT=polT.astype(np.float32),
                dmask=dmask.astype(bf),
            )
        )
    return in_maps


def kernel(x, policy, qkv_w, proj_w, proj_b):
    global LAST_RESULTS
    x = np.asarray(x, dtype=np.float32)
    policy = np.asarray(policy, dtype=np.float32)
    qkv_w = np.asarray(qkv_w, dtype=np.float32)
    proj_w = np.asarray(proj_w, dtype=np.float32)
    proj_b = np.asarray(proj_b, dtype=np.float32)

    in_maps = _pack_inputs(x, policy, qkv_w, proj_w, proj_b)

    nc = _get_nc()
    trace = os.environ.get("KERNEL_TRACE", "0") == "1"
    res = run_bass_kernel_spmd(
        nc,
        in_maps,
        core_ids=list(range(B)),
        trace=trace,
        trace_cores=list(range(B)) if trace else None,
        stitch_traces=False,
    )
    LAST_RESULTS = res
    return np.stack([res.results[b]["y"] for b in range(B)], axis=0)

